# revision 3
# baseline (speedup 1.0000x reference)
"""MoE (8 experts, top-2, shared expert) Trainium2 kernel.

Expert-parallel over 8 NeuronCores. The host performs only the dispatch
decision (top-2 expert ids -> compact per-expert token lists) and data
layout (transposes/gathers); all floating-point model math — router
logits, gates, expert SwiGLU, shared expert, and the cross-core combine
(ReduceScatter) — runs on device in fp32r matmuls with fp32 accumulation.
"""

import numpy as np

H = 1024          # hidden
I = 1408          # moe intermediate
E = 8             # experts == cores
T = 2048          # tokens (2*1024)
TOPK = 2
C = 640           # compact per-expert token capacity (max observed 540)
CH = 320          # ht token chunk (2 chunks; >=256 keeps fp32r at full rate)
ILOC = I // E     # 176: shared-expert intermediate slice per core
TSL = T // E      # 256: output token slice per core
KT = H // 128     # 8 contraction tiles over H
IT = I // 128     # 11 tiles over I
CT = C // 128     # 5 compact token tiles
SIP = (128, ILOC - 128)   # shared I-slice partition tiles: 128 + 48
NCORES = 8

_BUILD_CACHE = {}


def _build(reps=1):
    import concourse.bacc as bacc
    import concourse.bass as bass
    import concourse.mybir as mybir
    from concourse import tile
    from contextlib import ExitStack

    f32 = mybir.dt.float32
    f32r = mybir.dt.float32r
    i32 = mybir.dt.int32
    AF = mybir.ActivationFunctionType
    MUL = mybir.AluOpType.mult

    nc = bacc.Bacc("TRN2", target_bir_lowering=False, debug=False,
                   num_devices=NCORES)

    xg = nc.declare_dram_parameter("xg", [H, C], f32r, isOutput=False)
    xt = nc.declare_dram_parameter("xt", [H, T], f32r, isOutput=False)
    w1t = nc.declare_dram_parameter("w1t", [IT, H, 128], f32r, isOutput=False)
    w3t = nc.declare_dram_parameter("w3t", [IT, H, 128], f32r, isOutput=False)
    w2t = nc.declare_dram_parameter("w2t", [I, H], f32r, isOutput=False)
    s1t = nc.declare_dram_parameter("s1t", [H, ILOC], f32r, isOutput=False)
    s3t = nc.declare_dram_parameter("s3t", [H, ILOC], f32r, isOutput=False)
    s2t = nc.declare_dram_parameter("s2t", [ILOC, H], f32r, isOutput=False)
    rwe = nc.declare_dram_parameter("rwe", [H, 16], f32r, isOutput=False)
    idx = nc.declare_dram_parameter("idx", [C, 1], i32, isOutput=False)
    msk = nc.declare_dram_parameter("msk", [128, CT], f32, isOutput=False)
    out = nc.declare_dram_parameter("out", [TSL, H], f32, isOutput=True)

    acc = nc.dram_tensor("acc", [T, H], f32)
    rs_out = nc.dram_tensor("rs_out", [TSL, H], f32)

    with tile.TileContext(nc) as tc, ExitStack() as ctx:
        sres = ctx.enter_context(tc.tile_pool(name="sres", bufs=1))
        wstr = ctx.enter_context(tc.tile_pool(name="wstr", bufs=2))
        xstr = ctx.enter_context(tc.tile_pool(name="xstr", bufs=2))
        work = ctx.enter_context(tc.tile_pool(name="work", bufs=2))
        psA = ctx.enter_context(tc.tile_pool(name="psA", bufs=2, space="PSUM"))
        psB = ctx.enter_context(tc.tile_pool(name="psB", bufs=2, space="PSUM"))
        psY = ctx.enter_context(tc.tile_pool(name="psY", bufs=2, space="PSUM"))

        for _rep in range(reps):
            # ---- resident loads ----
            xg_sb = sres.tile([128, KT * C], f32r, tag="xg_sb", name="xg_sb")
            for k in range(KT):
                nc.sync.dma_start(xg_sb[:, k * C:(k + 1) * C],
                                  xg[k * 128:(k + 1) * 128, :])
            rwe_sb = sres.tile([128, KT * 16], f32r, tag="rwe_sb",
                               name="rwe_sb")
            nc.sync.dma_start(rwe_sb[:],
                              rwe.rearrange("(k p) o -> p k o", p=128))
            idx_sb = sres.tile([128, CT], i32, tag="idx_sb", name="idx_sb")
            nc.sync.dma_start(idx_sb[:],
                              idx.rearrange("(c p) o -> p c o", p=128))
            msk_sb = sres.tile([128, CT], f32, tag="msk_sb", name="msk_sb")
            nc.sync.dma_start(msk_sb[:], msk[:, :])
            # shared expert weights (packed along free dim)
            s13_sb = sres.tile([128, 2 * KT * ILOC], f32r, tag="s13_sb",
                               name="s13_sb")
            for k in range(KT):
                nc.sync.dma_start(s13_sb[:, k * ILOC:(k + 1) * ILOC],
                                  s1t[k * 128:(k + 1) * 128, :])
                nc.sync.dma_start(
                    s13_sb[:, (KT + k) * ILOC:(KT + k + 1) * ILOC],
                    s3t[k * 128:(k + 1) * 128, :])
            s2_sb = sres.tile([128, 2 * H], f32r, tag="s2_sb", name="s2_sb")
            nc.sync.dma_start(s2_sb[:, 0:H], s2t[0:128, :])
            nc.sync.dma_start(s2_sb[:SIP[1], H:2 * H], s2t[128:ILOC, :])

            # ---- Part A: logits for compact tokens -> gates ----
            gates_sb = sres.tile([128, CT], f32, tag="gates_sb",
                                 name="gates_sb")
            for ct in range(CT):
                psl = psY.tile([128, 512], f32, tag="y", name="psl",
                               space="PSUM")
                for k in range(KT):
                    nc.tensor.matmul(
                        psl[:, 0:16],
                        lhsT=xg_sb[:, k * C + ct * 128: k * C + (ct + 1) * 128],
                        rhs=rwe_sb[:, k * 16:(k + 1) * 16],
                        start=(k == 0), stop=(k == KT - 1))
                nc.vector.tensor_tensor(out=gates_sb[:, ct:ct + 1],
                                        in0=psl[:, 0:1],
                                        in1=msk_sb[:, ct:ct + 1], op=MUL)

            # ---- Part B: expert ht[I, C] = silu(w1@x) * (w3@x) ----
            ht_sb = sres.tile([128, IT * C], f32r, tag="ht_sb", name="ht_sb")
            for i in range(IT):
                w1b = wstr.tile([128, KT * 128], f32r, tag="w1b", name="w1b")
                nc.sync.dma_start(w1b[:],
                                  w1t[i].rearrange("(k p) m -> p k m", p=128))
                w3b = wstr.tile([128, KT * 128], f32r, tag="w3b", name="w3b")
                nc.sync.dma_start(w3b[:],
                                  w3t[i].rearrange("(k p) m -> p k m", p=128))
                for cc in range(C // CH):
                    psa = psA.tile([128, CH], f32, tag="a", name="psa",
                                   space="PSUM")
                    psb = psB.tile([128, CH], f32, tag="b", name="psb",
                                   space="PSUM")
                    for k in range(KT):
                        nc.tensor.matmul(
                            psa[:],
                            lhsT=w1b[:, k * 128:(k + 1) * 128],
                            rhs=xg_sb[:, k * C + cc * CH: k * C + (cc + 1) * CH],
                            start=(k == 0), stop=(k == KT - 1))
                    for k in range(KT):
                        nc.tensor.matmul(
                            psb[:],
                            lhsT=w3b[:, k * 128:(k + 1) * 128],
                            rhs=xg_sb[:, k * C + cc * CH: k * C + (cc + 1) * CH],
                            start=(k == 0), stop=(k == KT - 1))
                    sact = work.tile([128, CH], f32, tag="sact", name="sact")
                    nc.scalar.activation(sact[:], psa[:], AF.Silu)
                    nc.vector.tensor_tensor(
                        out=ht_sb[:, i * C + cc * CH: i * C + (cc + 1) * CH],
                        in0=sact[:], in1=psb[:], op=MUL)

            # ---- Part D: shared expert partial over ALL tokens -> acc ----
            TCH = 256
            for tt in range(T // TCH):
                xc = xstr.tile([128, KT * TCH], f32r, tag="xc", name="xc")
                for k in range(KT):
                    nc.sync.dma_start(
                        xc[:, k * TCH:(k + 1) * TCH],
                        xt[k * 128:(k + 1) * 128, tt * TCH:(tt + 1) * TCH])
                hts = xstr.tile([128, 2 * TCH], f32r, tag="hts", name="hts")
                for si in range(2):
                    sip = SIP[si]
                    psa = psA.tile([128, TCH], f32, tag="a", name="psa_s",
                                   space="PSUM")
                    psb = psB.tile([128, TCH], f32, tag="b", name="psb_s",
                                   space="PSUM")
                    for k in range(KT):
                        nc.tensor.matmul(
                            psa[:sip, :],
                            lhsT=s13_sb[:, k * ILOC + si * 128:
                                        k * ILOC + si * 128 + sip],
                            rhs=xc[:, k * TCH:(k + 1) * TCH],
                            start=(k == 0), stop=(k == KT - 1))
                    for k in range(KT):
                        nc.tensor.matmul(
                            psb[:sip, :],
                            lhsT=s13_sb[:, (KT + k) * ILOC + si * 128:
                                        (KT + k) * ILOC + si * 128 + sip],
                            rhs=xc[:, k * TCH:(k + 1) * TCH],
                            start=(k == 0), stop=(k == KT - 1))
                    sact = work.tile([128, TCH], f32, tag="sact_s",
                                     name="sact_s")
                    nc.scalar.activation(sact[:sip, :], psa[:sip, :], AF.Silu)
                    nc.vector.tensor_tensor(
                        out=hts[:sip, si * TCH:(si + 1) * TCH],
                        in0=sact[:sip, :], in1=psb[:sip, :], op=MUL)
                for q in range(TCH // 128):
                    ysb = work.tile([128, H], f32, tag="ysb", name="ysb")
                    for hh in range(2):
                        psy = psY.tile([128, 512], f32, tag="y", name="psy_s",
                                       space="PSUM")
                        nc.tensor.matmul(
                            psy[:],
                            lhsT=hts[:, q * 128:(q + 1) * 128],
                            rhs=s2_sb[:, hh * 512:(hh + 1) * 512],
                            start=True, stop=False)
                        nc.tensor.matmul(
                            psy[:],
                            lhsT=hts[:SIP[1], TCH + q * 128: TCH + (q + 1) * 128],
                            rhs=s2_sb[:SIP[1], H + hh * 512: H + (hh + 1) * 512],
                            start=False, stop=True)
                        nc.vector.tensor_copy(ysb[:, hh * 512:(hh + 1) * 512],
                                              psy[:])
                    trow = tt * (TCH // 128) + q
                    nc.sync.dma_start(acc[trow * 128:(trow + 1) * 128, :],
                                      ysb[:])

            # ---- Part C: expert y for compact tokens, combine into acc ----
            y_sb = sres.tile([128, CT * H], f32, tag="y_sb", name="y_sb")
            for hh in range(2):
                w2h = sres.tile([128, IT * 512], f32r, tag="w2h", name="w2h")
                for i in range(IT):
                    nc.sync.dma_start(
                        w2h[:, i * 512:(i + 1) * 512],
                        w2t[i * 128:(i + 1) * 128, hh * 512:(hh + 1) * 512])
                for ct in range(CT):
                    psy = psY.tile([128, 512], f32, tag="y", name="psy",
                                   space="PSUM")
                    for i in range(IT):
                        nc.tensor.matmul(
                            psy[:],
                            lhsT=ht_sb[:, i * C + ct * 128: i * C + (ct + 1) * 128],
                            rhs=w2h[:, i * 512:(i + 1) * 512],
                            start=(i == 0), stop=(i == IT - 1))
                    nc.scalar.activation(
                        y_sb[:, ct * H + hh * 512: ct * H + hh * 512 + 512],
                        psy[:], AF.Copy, scale=gates_sb[:, ct:ct + 1])
            shr_tiles = []
            for ct in range(CT):
                shr = work.tile([128, H], f32, tag=f"shr{ct}", name=f"shr{ct}", bufs=1)
                nc.gpsimd.indirect_dma_start(
                    out=shr[:], out_offset=None,
                    in_=acc[:, :],
                    in_offset=bass.IndirectOffsetOnAxis(
                        ap=idx_sb[:, ct:ct + 1], axis=0))
                shr_tiles.append(shr)
            for ct in range(CT):
                yfin = work.tile([128, H], f32, tag="yfin", name="yfin")
                nc.vector.tensor_add(yfin[:],
                                     y_sb[:, ct * H:(ct + 1) * H],
                                     shr_tiles[ct][:])
                nc.gpsimd.indirect_dma_start(
                    out=acc[:, :],
                    out_offset=bass.IndirectOffsetOnAxis(
                        ap=idx_sb[:, ct:ct + 1], axis=0),
                    in_=yfin[:], in_offset=None)

            # ---- Part E: cross-core combine + output ----
            nc.gpsimd.collective_compute(
                "ReduceScatter",
                mybir.AluOpType.add,
                replica_groups=[list(range(NCORES))],
                ins=[acc[:, :]],
                outs=[rs_out[:, :]],
            )
            for j in range(TSL // 128):
                ob = work.tile([128, H], f32, tag="ob", name="ob")
                nc.sync.dma_start(ob[:], rs_out[j * 128:(j + 1) * 128, :])
                nc.sync.dma_start(out[j * 128:(j + 1) * 128, :], ob[:])

    nc.finalize()
    return nc


def _get_nc(reps=1):
    if reps not in _BUILD_CACHE:
        _BUILD_CACHE[reps] = _build(reps)
    return _BUILD_CACHE[reps]


def _dispatch(x2, router_w):
    """Host-side sharding decision: per-expert compact token lists."""
    logits = x2 @ router_w.T                      # [T, E] fp32, dispatch only
    order = np.argsort(-logits, axis=1, kind="stable")[:, :TOPK]
    per_core = []
    all_rows = np.arange(T)
    for e in range(E):
        rows = all_rows[(order == e).any(axis=1)]
        ce = len(rows)
        assert ce <= C, f"expert {e} overflow: {ce} > {C}"
        unused = np.setdiff1d(all_rows, rows, assume_unique=True)
        pad = unused[:C - ce]
        idx_full = np.concatenate([rows, pad]).astype(np.int32)
        mask = (np.arange(C) < ce).astype(np.float32)
        per_core.append((idx_full, mask))
    return per_core


def kernel(x, router_w, w1, w2, w3, sw1, sw2, sw3):
    from concourse.bass_utils import run_bass_kernel_spmd

    in_dtype = x.dtype
    x2 = np.ascontiguousarray(x.reshape(T, H), dtype=np.float32)
    router_w = np.asarray(router_w, dtype=np.float32)
    nc = _get_nc()

    dispatch = _dispatch(x2, router_w)
    xt_host = np.ascontiguousarray(x2.T)

    in_maps = []
    for e in range(E):
        idx_full, mask = dispatch[e]
        xg_host = np.ascontiguousarray(x2[idx_full].T)          # [H, C]
        w1t_host = np.ascontiguousarray(
            np.asarray(w1[e], dtype=np.float32).reshape(IT, 128, H)
            .transpose(0, 2, 1))                                 # [IT, H, 128]
        w3t_host = np.ascontiguousarray(
            np.asarray(w3[e], dtype=np.float32).reshape(IT, 128, H)
            .transpose(0, 2, 1))
        w2t_host = np.ascontiguousarray(np.asarray(w2[e], np.float32).T)
        s1t_host = np.ascontiguousarray(
            np.asarray(sw1[e * ILOC:(e + 1) * ILOC, :], np.float32).T)
        s3t_host = np.ascontiguousarray(
            np.asarray(sw3[e * ILOC:(e + 1) * ILOC, :], np.float32).T)
        s2t_host = np.ascontiguousarray(
            np.asarray(sw2[:, e * ILOC:(e + 1) * ILOC], np.float32).T)
        in_maps.append({
            "xg": xg_host,
            "xt": xt_host,
            "w1t": w1t_host,
            "w3t": w3t_host,
            "w2t": w2t_host,
            "s1t": s1t_host,
            "s3t": s3t_host,
            "s2t": s2t_host,
            "rwe": np.ascontiguousarray(np.repeat(router_w[e].reshape(H, 1), 16, axis=1)),
            "idx": idx_full.reshape(C, 1),
            "msk": np.ascontiguousarray(mask.reshape(CT, 128).T),
        })

    res = run_bass_kernel_spmd(nc, in_maps, list(range(NCORES)))
    out = np.concatenate([res.results[i]["out"] for i in range(NCORES)],
                         axis=0)
    return out.reshape(x.shape).astype(in_dtype)
